# revision 1
# baseline (speedup 1.0000x reference)
"""Causal self-attention (B=4, S=2048, D=1024, H=16) on 8 TRN2 NeuronCores.

Sharding (tensor-parallel on heads + data-parallel on batch):
  core c -> batch c//2, head-half c%2 (8 of 16 heads).
  Wq/Wk/Wv column-split, Wo row-split; the two partial outputs per batch are
  summed on the host (+ bo), which is the row-parallel unshard.

Per-core Bass/Tile program (matmul operands bf16, psum/softmax fp32):
  phase A: qT/kT feature-major projections (4-moving-block stationary chains);
           v token-major with a per-head ones column, emitted per-superblock
           interleaved with attention to keep the PE stream dense.
  phase B: per head / 512-query superblock / 128-key tile:
           scoresT = k_j @ q_blk.T (keys on partitions, two heads on disjoint
           PE row groups), additive triangular mask on the diagonal boundary
           subtile, dead columns skipped in scores, exp and PV
           (no max subtraction: scores ~ N(0,1)); PV accumulation with the
           ones column producing sumexp in row 64; reciprocal broadcast via a
           K=1 matmul; PV emission software-pipelined one key tile behind
           scores to hide the exp latency.
  phase C: out_partial = attnT.T @ Wo_rows (stationary reused across the two
           output column blocks).
"""

from contextlib import ExitStack

import numpy as np
import ml_dtypes

import concourse.bass as bass
import concourse.bacc as bacc
import concourse.tile as tile
import concourse.mybir as mybir

F32 = mybir.dt.float32
F32R = mybir.dt.float32r
BF16 = mybir.dt.bfloat16
NEG = -30000.0  # additive mask; must stay finite-representable in bf16 paths


def r(ap):
    return ap.bitcast(F32R)


def build_core_program(S=2048, D=1024, HC=8, DH=64, SQ=512, mm_dt=BF16,
                       xt_bufs=2, qk_psum_bufs=4, probs_bufs=6):
    """Build the per-core Bass program (SPMD: same program, different data).
    mm_dt: dtype of matmul operands (BF16 or F32R). When BF16, the host must
    pass xT/wqk/wv/wo as bfloat16 arrays."""
    DQ = HC * DH              # head-slice width (512)
    DK = D // 128             # contraction tiles for projections (8)
    DQN = DQ // 128           # head-pair tiles (4)
    NSB = S // SQ             # query superblocks (4)
    NTT = S // 128            # token tiles (16)
    NOUT = min(512, D)        # output-proj free width
    NOB = D // NOUT           # output-proj col blocks (2)
    assert DQ % 128 == 0 and S % SQ == 0 and SQ % 128 == 0 and D % 128 == 0
    assert (S // SQ) % 2 == 0

    bf = mm_dt == BF16
    in_dt = BF16 if bf else F32

    def m(ap):
        # bitcast for f32->f32r reinterpretation; no-op for bf16 tiles
        return ap if bf else ap.bitcast(F32R)

    nc = bacc.Bacc("TRN2", target_bir_lowering=False, debug=False)

    xT = nc.dram_tensor("xT", [D, S], in_dt, kind="ExternalInput").ap()
    wqk = nc.dram_tensor("wqk", [D, 2 * DQ], in_dt, kind="ExternalInput").ap()
    wv = nc.dram_tensor("wv", [D, DQ], in_dt, kind="ExternalInput").ap()
    wo = nc.dram_tensor("wo", [DQ, D], in_dt, kind="ExternalInput").ap()
    bqk = nc.dram_tensor("bqk", [2 * DQ], F32, kind="ExternalInput").ap()
    bv = nc.dram_tensor("bv", [DQ], F32, kind="ExternalInput").ap()
    out = nc.dram_tensor("out", [S, D], F32, kind="ExternalOutput").ap()

    with tile.TileContext(nc) as tc, ExitStack() as ctx:
        ctx.enter_context(nc.allow_low_precision(
            reason="low-precision matmul operands; accumulation stays fp32"))
        const = ctx.enter_context(tc.tile_pool(name="const", bufs=1))
        big = ctx.enter_context(tc.tile_pool(name="big", bufs=1))
        stream = ctx.enter_context(tc.tile_pool(name="stream", bufs=1))
        psum = ctx.enter_context(tc.tile_pool(name="psum", bufs=1, space="PSUM"))

        # ---- constants ----
        # triangular mask [128,128]: 0 where p <= f else NEG (boundary subtile)
        tri = const.tile([128, 128], F32)
        nc.vector.memset(tri[:], 0.0)
        nc.gpsimd.affine_select(
            out=tri[:], in_=tri[:], compare_op=mybir.AluOpType.is_ge,
            fill=NEG, base=0, channel_multiplier=-1, pattern=[[1, 128]],
        )
        ones128f = const.tile([1, 128], F32)
        nc.vector.memset(ones128f[:], 1.0)
        ones64r = const.tile([1, 64], F32R)
        nc.vector.tensor_copy(ones64r[:], ones128f[:, 0:64])
        ones128r = const.tile([1, 128], F32R)
        nc.vector.tensor_copy(ones128r[:], ones128f[:])
        ones_hc = const.tile([128, HC], F32)
        nc.vector.memset(ones_hc[:], 1.0)

        # biases: bqk as [128, 2*DQN] (column t = dout tile t), bv broadcast
        bqk_sb = const.tile([128, 2 * DQN], F32)
        nc.sync.dma_start(bqk_sb[:], bqk.rearrange("(t p) -> p t", p=128))
        bv_rowf = const.tile([1, DQ], F32)
        nc.sync.dma_start(bv_rowf[:], bv.rearrange("(a d) -> a d", a=1))
        bv_row = const.tile([1, DQ], F32R)
        nc.vector.tensor_copy(bv_row[:], bv_rowf[:])
        bv_bc = const.tile([128, DQ], F32)
        bv_ps = psum.tile([128, DQ], F32, tag="v", bufs=2)
        nc.tensor.matmul(bv_ps[:], r(ones128r[:]), r(bv_row[:]),
                         start=True, stop=True)
        nc.scalar.copy(bv_bc[:], bv_ps[:])

        # ---- big resident tensors ----
        kT = big.tile([128, DQN, S], mm_dt)     # [pair 2x64 rows, tokens]
        qT = big.tile([128, DQN, S], mm_dt)
        v_aug = big.tile([128, NTT, HC * 65], mm_dt)
        wv_sb = big.tile([128, DK, DQ], mm_dt)
        wo_sb = big.tile([128, DQN, D], mm_dt)
        xt_all = big.tile([128, DK, S], mm_dt)

        for kt in range(DK):
            nc.sync.dma_start(xt_all[:, kt, :], m(xT[128 * kt:128 * (kt + 1), :]))

        # ===== phase A-qk: all projections, 4-moving-block stationary chains
        for dt in range(2 * DQN):
            wdt = stream.tile([128, DK, 128], mm_dt, tag="wdt", bufs=3)
            for kt in range(DK):
                nc.gpsimd.dma_start(
                    wdt[:, kt, :],
                    m(wqk[128 * kt:128 * (kt + 1), 128 * dt:128 * (dt + 1)]))
            pss = [psum.tile([128, SQ], F32, tag="qk", bufs=qk_psum_bufs,
                             name=f"pss_{dt}_{tb}") for tb in range(NSB)]
            for kt in range(DK):
                for tb in range(NSB):
                    nc.tensor.matmul(
                        pss[tb][:], m(wdt[:, kt, :]),
                        m(xt_all[:, kt, tb * SQ:(tb + 1) * SQ]),
                        start=(kt == 0), stop=(kt == DK - 1))
            is_q = dt < DQN
            hp = dt % DQN
            dest = qT if is_q else kT
            for tb in range(NSB):
                nc.scalar.activation(
                    dest[:, hp, tb * SQ:(tb + 1) * SQ], pss[tb][:],
                    mybir.ActivationFunctionType.Identity,
                    bias=bqk_sb[:, dt:dt + 1],
                    scale=0.125 if is_q else 1.0)

        def emit_v_group(blk):
            # v projection for token tiles of one superblock (token-stationary)
            for tt in range(blk * (SQ // 128), (blk + 1) * (SQ // 128)):
                psv = psum.tile([128, DQ], F32, tag="v", bufs=2,
                                name=f"psv_{tt}")
                for kt in range(DK):
                    nc.tensor.matmul(
                        psv[:], m(xt_all[:, kt, 128 * tt:128 * (tt + 1)]),
                        m(wv_sb[:, kt, :]),
                        start=(kt == 0), stop=(kt == DK - 1))
                va = v_aug[:, tt, :].rearrange("p (h c) -> p h c", h=HC)
                nc.vector.tensor_tensor(
                    va[:, :, 0:64], psv[:].rearrange("p (h c) -> p h c", h=HC),
                    bv_bc[:].rearrange("p (h c) -> p h c", h=HC),
                    op=mybir.AluOpType.add)
                nc.vector.tensor_copy(va[:, :, 64:65], ones_hc[:, :, None])

        for kt in range(DK):
            nc.gpsimd.dma_start(wv_sb[:, kt, :],
                                m(wv[128 * kt:128 * (kt + 1), :]))
        emit_v_group(0)
        for p4 in range(DQN):
            nc.gpsimd.dma_start(wo_sb[:, p4, :],
                                m(wo[128 * p4:128 * (p4 + 1), :]))

        for i in range(NSB):
            # ===== phase B: attention for superblock i =====================
            ND = SQ // 128
            NJ = ND * (i + 1)
            attnT = stream.tile([128, DQN, SQ], mm_dt, tag="attnT", bufs=2,
                                name=f"at_{i}")
            pending = [None]  # deferred (bc matmul + normalize) of prev hp
            for hp in range(DQN):
                pva = psum.tile([65, SQ], F32, tag="v", bufs=2,
                                name=f"pv_{i}_{hp}_0")
                pvb = psum.tile([65, SQ], F32, tag="v", bufs=2,
                                name=f"pv_{i}_{hp}_1")
                pvs = (pva, pvb)
                pend = None
                for j in range(NJ):
                    jj = j - ND * i
                    f0 = max(0, 128 * jj)
                    scs, prbs = [], []
                    for hh in range(2):
                        p0, p1 = 64 * hh, 64 * hh + 64
                        sc = psum.tile([128, SQ], F32, tag="qk",
                                       bufs=qk_psum_bufs,
                                       name=f"sc_{i}_{hp}_{j}_{hh}")
                        nc.tensor.matmul(
                            sc[:, f0:],
                            m(kT[p0:p1, hp, 128 * j:128 * (j + 1)]),
                            m(qT[p0:p1, hp, i * SQ + f0:(i + 1) * SQ]),
                            start=True, stop=True,
                            tile_position=(64 * hh, 0))
                        scs.append(sc)
                    if j == 1 and pending[0] is not None:
                        pending[0]()
                        pending[0] = None
                    for hh in range(2):
                        if jj >= 0:
                            nc.vector.tensor_tensor(
                                scs[hh][:, f0:f0 + 128],
                                scs[hh][:, f0:f0 + 128], tri[:],
                                op=mybir.AluOpType.add)
                        probs = stream.tile([128, SQ], mm_dt, tag="probs",
                                            bufs=probs_bufs,
                                            name=f"pr_{i}_{hp}_{j}_{hh}")
                        nc.scalar.activation(
                            probs[:, f0:], scs[hh][:, f0:],
                            mybir.ActivationFunctionType.Exp)
                        prbs.append(probs)
                    if pend is not None:
                        pprbs, pf0, pj = pend
                        for hh in range(2):
                            h = 2 * hp + hh
                            nc.tensor.matmul(
                                pvs[hh][:, pf0:],
                                m(v_aug[:, pj, 65 * h:65 * h + 65]),
                                m(pprbs[hh][:, pf0:]),
                                start=(pj == 0), stop=(pj == NJ - 1))
                    pend = (prbs, f0, j)
                pprbs, pf0, pj = pend
                for hh in range(2):
                    h = 2 * hp + hh
                    nc.tensor.matmul(
                        pvs[hh][:, pf0:],
                        m(v_aug[:, pj, 65 * h:65 * h + 65]),
                        m(pprbs[hh][:, pf0:]),
                        start=(pj == 0), stop=(pj == NJ - 1))
                recips = []
                for hh in range(2):
                    recip = stream.tile([1, SQ], F32R, tag="recip", bufs=4,
                                        name=f"rc_{i}_{hp}_{hh}")
                    nc.vector.reciprocal(recip[:], pvs[hh][64:65, :])
                    recips.append(recip)

                def make_norm(pvs=pvs, recips=recips, hp=hp, at=attnT, ii=i):
                    def emit():
                        for hh in range(2):
                            bc = psum.tile([64, SQ], F32, tag="out", bufs=2,
                                           name=f"bc_{ii}_{hp}_{hh}")
                            nc.tensor.matmul(bc[:], r(ones64r[:]),
                                             r(recips[hh][:]),
                                             start=True, stop=True)
                            bc_sb = stream.tile([64, SQ], F32, tag="bcs",
                                                bufs=2,
                                                name=f"bs_{ii}_{hp}_{hh}")
                            nc.vector.tensor_copy(bc_sb[:], bc[:])
                            if hh == 0:
                                nc.vector.tensor_tensor(
                                    at[0:64, hp, :],
                                    pvs[hh][0:64, :], bc_sb[:],
                                    op=mybir.AluOpType.mult)
                            else:
                                stage = stream.tile([64, SQ], mm_dt,
                                                    tag="stage", bufs=2,
                                                    name=f"st_{ii}_{hp}_{hh}")
                                nc.vector.tensor_tensor(
                                    stage[:], pvs[hh][0:64, :], bc_sb[:],
                                    op=mybir.AluOpType.mult)
                                nc.sync.dma_start(at[64:128, hp, :], stage[:])
                    return emit

                pending[0] = make_norm()

            if pending[0] is not None:
                pending[0]()
                pending[0] = None
            if i + 1 < NSB:
                emit_v_group(i + 1)

            # ===== phase C: output projection for superblock i ============
            for mm_ in range(SQ // 128):
                tt = i * (SQ // 128) + mm_
                pos = [psum.tile([128, NOUT], F32, tag="out", bufs=2,
                                 name=f"po_{tt}_{nb}") for nb in range(NOB)]
                for p4 in range(DQN):
                    for nb in range(NOB):
                        nc.tensor.matmul(
                            pos[nb][:],
                            m(attnT[:, p4, 128 * mm_:128 * (mm_ + 1)]),
                            m(wo_sb[:, p4, nb * NOUT:(nb + 1) * NOUT]),
                            start=(p4 == 0), stop=(p4 == DQN - 1))
                for nb in range(NOB):
                    osb = stream.tile([128, NOUT], F32, tag="osb", bufs=3,
                                      name=f"ob_{tt}_{nb}")
                    nc.vector.tensor_copy(osb[:], pos[nb][:])
                    nc.sync.dma_start(
                        out[128 * tt:128 * (tt + 1),
                            nb * NOUT:(nb + 1) * NOUT], osb[:])

    nc.compile()
    return nc

B, S, D, H = 4, 2048, 1024, 16
N_CORES = 8

_CACHED = {}


def _make_core_inputs(x, Wq, bq, Wk, bk, Wv, bv, Wo):
    DQ = D // 2

    def cast(a):
        return np.ascontiguousarray(a).astype(ml_dtypes.bfloat16)

    xTs = [cast(x[b].T) for b in range(B)]
    in_maps = []
    for c in range(N_CORES):
        b, hf = c // 2, c % 2
        sl = slice(hf * DQ, (hf + 1) * DQ)
        in_maps.append({
            "xT": xTs[b],
            "wqk": cast(np.concatenate([Wq[:, sl], Wk[:, sl]], axis=1)),
            "wv": cast(Wv[:, sl]),
            "wo": cast(Wo[sl, :]),
            "bqk": np.ascontiguousarray(
                np.concatenate([0.125 * bq[sl], bk[sl]])).astype(np.float32),
            "bv": np.ascontiguousarray(bv[sl]).astype(np.float32),
        })
    return in_maps


def kernel(x, Wq, bq, Wk, bk, Wv, bv, Wo, bo):
    import tempfile
    from concourse import bass_utils

    x = np.asarray(x, dtype=np.float32)
    Wq = np.asarray(Wq, dtype=np.float32)
    bq = np.asarray(bq, dtype=np.float32)
    Wk = np.asarray(Wk, dtype=np.float32)
    bk = np.asarray(bk, dtype=np.float32)
    Wv = np.asarray(Wv, dtype=np.float32)
    bv = np.asarray(bv, dtype=np.float32)
    Wo = np.asarray(Wo, dtype=np.float32)
    bo = np.asarray(bo, dtype=np.float32)

    if "nc" not in _CACHED:
        _CACHED["nc"] = build_core_program(S=S, D=D, HC=H // 2)
    nc = _CACHED["nc"]

    in_maps = _make_core_inputs(x, Wq, bq, Wk, bk, Wv, bv, Wo)
    res = bass_utils.run_bass_kernel_spmd(
        nc, in_maps, core_ids=list(range(N_CORES)),
        tmpdir=tempfile.mkdtemp(prefix="bass_attn_"))

    out = np.empty((B, S, D), dtype=np.float32)
    for b in range(B):
        out[b] = res.results[2 * b]["out"] + res.results[2 * b + 1]["out"] + bo
    return out



# revision 8
# speedup vs baseline: 1.4140x; 1.4140x over previous
"""Causal self-attention (B=4, S=2048, D=1024, H=16) on 8 TRN2 NeuronCores.

Sharding (tensor-parallel on heads + data-parallel on batch):
  core c -> batch c//2, head-half c%2 (8 of 16 heads).
  Wq/Wk/Wv column-split, Wo row-split; the two partial outputs per batch are
  summed on the host (+ bo), which is the row-parallel unshard.

Per-core Bass/Tile program (matmul operands bf16, psum/softmax fp32), built
around keeping the PE stream dense (HAM stays at K=8/8) and the ScalarE exp
stream saturated:

  prologue: q/k projections for head-pair 0 and v for token tiles 0..7.
  main loop (hp outer, superblock i inner, key tile j innermost):
    scores for both heads of the pair land in one 2-bank psum tile
    ([128, 1024], row-group tile_position packing); ONE exp activation per
    key tile covers both heads; diagonal-tile causal masking is a single
    GpSimd affine_select that zeroes the upper triangle of probs (garbage
    from the skipped dead columns is zeroed by the same select); PV (ones
    column producing sumexp in row 64) runs one key tile behind the exp.
    PE idle slots during the ScalarE-paced stretches are filled with v
    projections (hp 0), the next head-pair's q/k projections (hp 0..2) and
    the output projection for completed superblocks (hp 3).
  normalize (deferred one (hp, i) unit): reciprocal_approx_fast on the
    sumexp rows, GpSimd partition_broadcast, DVE multiply into attnT.
  phase C: out_partial = attnT.T @ Wo_rows per 128-token tile.
"""

from collections import deque
from contextlib import ExitStack

import numpy as np
import ml_dtypes

import concourse.bass as bass
import concourse.bacc as bacc
import concourse.tile as tile
import concourse.mybir as mybir

F32 = mybir.dt.float32
F32R = mybir.dt.float32r
BF16 = mybir.dt.bfloat16


def build_core_program(S=2048, D=1024, HC=8, DH=64, SQ=512):
    """Build the per-core Bass program (SPMD: same program, different data).
    The host must pass xT/wqk/wv/wo as bfloat16 arrays."""
    DQ = HC * DH              # head-slice width (512)
    DK = D // 128             # contraction tiles for projections (8)
    DQN = DQ // 128           # head-pair count (4)
    NSB = S // SQ             # query superblocks (4)
    NTT = S // 128            # token tiles (16)
    ND = SQ // 128            # key tiles per superblock (4)
    assert DQ % 128 == 0 and S % SQ == 0 and SQ % 128 == 0 and D % 128 == 0

    nc = bacc.Bacc("TRN2", target_bir_lowering=False, debug=False)

    xT = nc.dram_tensor("xT", [D, S], BF16, kind="ExternalInput").ap()
    wqk = nc.dram_tensor("wqk", [D, 2 * DQ], BF16, kind="ExternalInput").ap()
    wv = nc.dram_tensor("wv", [D, DQ], BF16, kind="ExternalInput").ap()
    wo = nc.dram_tensor("wo", [DQ, D], BF16, kind="ExternalInput").ap()
    bqk = nc.dram_tensor("bqk", [2 * DQ], F32, kind="ExternalInput").ap()
    bv = nc.dram_tensor("bv", [DQ], F32, kind="ExternalInput").ap()
    out = nc.dram_tensor("out", [S, D], F32, kind="ExternalOutput").ap()

    with tile.TileContext(nc) as tc, ExitStack() as ctx:
        ctx.enter_context(nc.allow_low_precision(
            reason="low-precision matmul operands; accumulation stays fp32"))
        const = ctx.enter_context(tc.tile_pool(name="const", bufs=1))
        big = ctx.enter_context(tc.tile_pool(name="big", bufs=1))
        stream = ctx.enter_context(tc.tile_pool(name="stream", bufs=1))
        psum = ctx.enter_context(tc.tile_pool(name="psum", bufs=1, space="PSUM"))

        # ---- constants ----
        ones_hc = const.tile([128, HC], F32)
        nc.vector.memset(ones_hc[:], 1.0)

        # biases: bqk as [128, 2*DQN] (column t = dout tile t), bv broadcast
        bqk_sb = const.tile([128, 2 * DQN], F32)
        nc.sync.dma_start(bqk_sb[:], bqk.rearrange("(t p) -> p t", p=128))
        bv_rowf = const.tile([1, DQ], F32)
        nc.sync.dma_start(bv_rowf[:], bv.rearrange("(a d) -> a d", a=1))
        bv_bc = const.tile([128, DQ], F32)
        nc.gpsimd.partition_broadcast(bv_bc[:], bv_rowf[:])

        # ---- big resident tensors ----
        xt_all = big.tile([128, DK, S], BF16)
        wqk_sb = big.tile([128, DK, 2 * DQ], BF16)
        wv_sb = big.tile([128, DK, DQ], BF16)
        wo_sb = big.tile([128, DQN, D], BF16)
        kT = big.tile([128, DQN, S], BF16)      # [pair 2x64 rows, tokens]
        qT = big.tile([128, DQN, S], BF16)
        v_aug = big.tile([128, NTT, HC * 65], BF16)
        attnT = big.tile([128, DQN, S], BF16)

        for kt in range(DK):
            nc.sync.dma_start(xt_all[:, kt, :], xT[128 * kt:128 * (kt + 1), :])
            nc.sync.dma_start(wqk_sb[:, kt, :], wqk[128 * kt:128 * (kt + 1), :])
            nc.sync.dma_start(wv_sb[:, kt, :], wv[128 * kt:128 * (kt + 1), :])
        for p4 in range(DQN):
            nc.sync.dma_start(wo_sb[:, p4, :], wo[128 * p4:128 * (p4 + 1), :])

        # ---- work units (emitted inline or as fillers) -------------------
        def proj_unit(dt, pair):
            # q/k projection: out-dim block dt, token superblocks 2p, 2p+1
            def emit():
                pss = [psum.tile([128, SQ], F32, tag="misc", bufs=2,
                                 name=f"pp_{dt}_{pair}_{t}") for t in range(2)]
                for kt in range(DK):
                    for t in range(2):
                        nc.tensor.matmul(
                            pss[t][:],
                            wqk_sb[:, kt, 128 * dt:128 * (dt + 1)],
                            xt_all[:, kt, (2 * pair + t) * SQ:
                                   (2 * pair + t + 1) * SQ],
                            start=(kt == 0), stop=(kt == DK - 1))
                is_q = dt < DQN
                hp = dt % DQN
                dest = qT if is_q else kT
                for t in range(2):
                    tb = 2 * pair + t
                    nc.vector.tensor_scalar(
                        dest[:, hp, tb * SQ:(tb + 1) * SQ], pss[t][:],
                        0.125 if is_q else 1.0, bqk_sb[:, dt:dt + 1],
                        op0=mybir.AluOpType.mult, op1=mybir.AluOpType.add)
            return emit

        def v_unit(tt):
            # v projection for one 128-token tile (token-stationary)
            def emit():
                psv = psum.tile([128, DQ], F32, tag="misc", bufs=2,
                                name=f"pv_{tt}")
                for kt in range(DK):
                    nc.tensor.matmul(
                        psv[:], xt_all[:, kt, 128 * tt:128 * (tt + 1)],
                        wv_sb[:, kt, :],
                        start=(kt == 0), stop=(kt == DK - 1))
                va = v_aug[:, tt, :].rearrange("p (h c) -> p h c", h=HC)
                nc.vector.tensor_tensor(
                    va[:, :, 0:64], psv[:].rearrange("p (h c) -> p h c", h=HC),
                    bv_bc[:].rearrange("p (h c) -> p h c", h=HC),
                    op=mybir.AluOpType.add)
                nc.vector.tensor_copy(va[:, :, 64:65], ones_hc[:, :, None])
            return emit

        def phase_c_unit(tt):
            # output projection for one 128-token tile
            def emit():
                pos = [psum.tile([128, SQ], F32, tag="misc", bufs=2,
                                 name=f"po_{tt}_{nb}") for nb in range(2)]
                for p4 in range(DQN):
                    for nb in range(2):
                        nc.tensor.matmul(
                            pos[nb][:],
                            attnT[:, p4, 128 * tt:128 * (tt + 1)],
                            wo_sb[:, p4, nb * SQ:(nb + 1) * SQ],
                            start=(p4 == 0), stop=(p4 == DQN - 1))
                for nb in range(2):
                    osb = stream.tile([128, SQ], F32, tag="osb", bufs=3,
                                      name=f"ob_{tt}_{nb}")
                    nc.vector.tensor_copy(osb[:], pos[nb][:])
                    nc.sync.dma_start(
                        out[128 * tt:128 * (tt + 1),
                            nb * SQ:(nb + 1) * SQ], osb[:])
            return emit

        def make_norm(hp, i, pva, pvb):
            # deferred: 1/sumexp, partition-broadcast, scale into attnT
            def emit():
                for hh, pv in ((0, pva), (1, pvb)):
                    # custom-DVE ops mishandle non-zero partition offsets:
                    # evacuate the sumexp row to a partition-0 SBUF tile with
                    # a standard copy before reciprocal_approx_fast
                    se = stream.tile([1, SQ], F32, tag="se", bufs=4,
                                     name=f"se_{hp}_{i}_{hh}")
                    nc.vector.tensor_copy(se[:], pv[64:65, :])
                    rc = stream.tile([1, SQ], F32, tag="recip", bufs=4,
                                     name=f"rc_{hp}_{i}_{hh}")
                    nc.vector.reciprocal_approx_fast(rc[:], se[:])
                    bc = stream.tile([64, SQ], F32, tag="bc", bufs=4,
                                     name=f"bn_{hp}_{i}_{hh}")
                    nc.gpsimd.partition_broadcast(bc[:], rc[:])
                    if hh == 0:
                        nc.vector.tensor_tensor(
                            attnT[0:64, hp, i * SQ:(i + 1) * SQ],
                            pv[0:64, :], bc[:], op=mybir.AluOpType.mult)
                    else:
                        stage = stream.tile([64, SQ], BF16, tag="stage",
                                            bufs=2, name=f"st_{hp}_{i}")
                        nc.vector.tensor_tensor(
                            stage[:], pv[0:64, :], bc[:],
                            op=mybir.AluOpType.mult)
                        nc.sync.dma_start(
                            attnT[64:128, hp, i * SQ:(i + 1) * SQ], stage[:])
            return emit

        # ---- prologue: head-pair 0 projections + first half of v ---------
        for pair in range(2):
            proj_unit(0, pair)()        # q, head pair 0
            proj_unit(DQN, pair)()      # k, head pair 0
        for tt in range(NTT // 2):
            v_unit(tt)()

        # ---- main loop ----------------------------------------------------
        fillers = deque()
        fillers.extend(v_unit(tt) for tt in range(NTT // 2, NTT))
        pend_norm = [None]

        for hp in range(DQN):
            if hp < DQN - 1:
                ndt = hp + 1
                for pair in range(2):
                    fillers.append(proj_unit(ndt, pair))
                    fillers.append(proj_unit(DQN + ndt, pair))
            for i in range(NSB):
                if hp == DQN - 1 and i >= 1:
                    # all heads' attnT for superblock i-1 complete after the
                    # deferred normalize emitted at j==1 below
                    fillers.extend(
                        phase_c_unit((i - 1) * ND + m) for m in range(ND))
                NJ = ND * (i + 1)
                pva = psum.tile([65, SQ], F32, tag="pv", bufs=2,
                                name=f"pa_{hp}_{i}")
                pvb = psum.tile([65, SQ], F32, tag="pv", bufs=2,
                                name=f"pb_{hp}_{i}")
                pend_pv = None
                for j in range(NJ):
                    jj = j - ND * i
                    f0 = max(0, 128 * jj)
                    sc = psum.tile([128, 2 * SQ], F32, tag="sc", bufs=2,
                                   name=f"sc_{hp}_{i}_{j}")
                    probs = stream.tile([128, 2 * SQ], BF16, tag="probs",
                                        bufs=4, name=f"pr_{hp}_{i}_{j}")
                    for hh in range(2):
                        p0 = 64 * hh
                        nc.tensor.matmul(
                            sc[:, hh * SQ:(hh + 1) * SQ],
                            kT[p0:p0 + 64, hp, 128 * j:128 * (j + 1)],
                            qT[p0:p0 + 64, hp, i * SQ:(i + 1) * SQ],
                            start=True, stop=True,
                            tile_position=(p0, 0))
                    nc.scalar.activation(
                        probs[:], sc[:],
                        mybir.ActivationFunctionType.Exp)
                    if jj >= 0:
                        # zero probs where query < key; only the 128-wide
                        # boundary subtile matters — PV reads probs[:, f0:],
                        # so columns left of f0 are never consumed
                        for hh in range(2):
                            pr = probs[:, hh * SQ + f0:hh * SQ + f0 + 128]
                            nc.gpsimd.affine_select(
                                out=pr, in_=pr,
                                compare_op=mybir.AluOpType.is_ge,
                                fill=0.0, base=0, channel_multiplier=-1,
                                pattern=[[1, 128]])
                    if j == 1 and pend_norm[0] is not None:
                        pend_norm[0]()
                        pend_norm[0] = None
                    if pend_pv is not None:
                        pj, pf0, pprobs = pend_pv
                        for hh, pv in ((0, pva), (1, pvb)):
                            h = 2 * hp + hh
                            nc.tensor.matmul(
                                pv[:, pf0:],
                                v_aug[:, pj, 65 * h:65 * h + 65],
                                pprobs[:, hh * SQ + pf0:(hh + 1) * SQ],
                                start=(pj == 0), stop=(pj == NJ - 1))
                    pend_pv = (j, f0, probs)
                    if fillers and (hp == 0 or j % 3 == 2):
                        fillers.popleft()()
                pj, pf0, pprobs = pend_pv
                for hh, pv in ((0, pva), (1, pvb)):
                    h = 2 * hp + hh
                    nc.tensor.matmul(
                        pv[:, pf0:],
                        v_aug[:, pj, 65 * h:65 * h + 65],
                        pprobs[:, hh * SQ + pf0:(hh + 1) * SQ],
                        start=(pj == 0), stop=(pj == NJ - 1))
                pend_norm[0] = make_norm(hp, i, pva, pvb)

        # ---- tail: last normalize, remaining fillers, last superblock ----
        if pend_norm[0] is not None:
            pend_norm[0]()
            pend_norm[0] = None
        while fillers:
            fillers.popleft()()
        for m in range(ND):
            phase_c_unit((NSB - 1) * ND + m)()

    nc.compile()
    return nc


B, S, D, H = 4, 2048, 1024, 16
N_CORES = 8

_CACHED = {}


def _make_core_inputs(x, Wq, bq, Wk, bk, Wv, bv, Wo):
    DQ = D // 2

    def cast(a):
        return np.ascontiguousarray(a).astype(ml_dtypes.bfloat16)

    xTs = [cast(x[b].T) for b in range(B)]
    in_maps = []
    for c in range(N_CORES):
        b, hf = c // 2, c % 2
        sl = slice(hf * DQ, (hf + 1) * DQ)
        in_maps.append({
            "xT": xTs[b],
            "wqk": cast(np.concatenate([Wq[:, sl], Wk[:, sl]], axis=1)),
            "wv": cast(Wv[:, sl]),
            "wo": cast(Wo[sl, :]),
            "bqk": np.ascontiguousarray(
                np.concatenate([0.125 * bq[sl], bk[sl]])).astype(np.float32),
            "bv": np.ascontiguousarray(bv[sl]).astype(np.float32),
        })
    return in_maps


def kernel(x, Wq, bq, Wk, bk, Wv, bv, Wo, bo):
    import tempfile
    from concourse import bass_utils

    x = np.asarray(x, dtype=np.float32)
    Wq = np.asarray(Wq, dtype=np.float32)
    bq = np.asarray(bq, dtype=np.float32)
    Wk = np.asarray(Wk, dtype=np.float32)
    bk = np.asarray(bk, dtype=np.float32)
    Wv = np.asarray(Wv, dtype=np.float32)
    bv = np.asarray(bv, dtype=np.float32)
    Wo = np.asarray(Wo, dtype=np.float32)
    bo = np.asarray(bo, dtype=np.float32)

    if "nc" not in _CACHED:
        _CACHED["nc"] = build_core_program(S=S, D=D, HC=H // 2)
    nc = _CACHED["nc"]

    in_maps = _make_core_inputs(x, Wq, bq, Wk, bk, Wv, bv, Wo)
    res = bass_utils.run_bass_kernel_spmd(
        nc, in_maps, core_ids=list(range(N_CORES)),
        tmpdir=tempfile.mkdtemp(prefix="bass_attn_"))

    out = np.empty((B, S, D), dtype=np.float32)
    for b in range(B):
        out[b] = res.results[2 * b]["out"] + res.results[2 * b + 1]["out"] + bo
    return out
